# revision 10
# baseline (speedup 1.0000x reference)
"""Cumulative LayerNorm Trainium2 kernel.

For each step k, normalize inputs[:, k] by the mean/var of the prefix
inputs[:, :k+1] over both time and feature axes, then scale/shift by
gamma/beta.

Sharding: data-parallel over batch B=8 across 8 NeuronCores (one batch
element per core); gamma/beta replicated.

Per-core algorithm (K=8192 steps, H=512 features):
  - 64 blocks of [128 steps x 512 feat].  bn_stats/bn_aggr give per-step
    mean m_k and var v_k over H; e_k = v_k + m_k^2 = E[x^2]_k.
  - cumsum over k of m and e:  within-block prefix via a triangular
    matmul (contracts the partition axis), cross-block offsets via a
    tensor_tensor_scan over the block-totals row, broadcast back across
    partitions with a rank-1 ones matmul accumulated into the same PSUM.
  - mean_k = cumsum(m)[k] / (k+1);  q_k = cumsum(e)[k] / (k+1);
    var_k = q_k - mean_k^2;  istd = 1/sqrt(var_k + eps).
  - out = gamma * (x - mean) * istd (+ beta), fused as a per-partition
    tensor_scalar plus a broadcast tensor_tensor multiply.
"""

import sys

if "/opt/trn_rl_repo" not in sys.path:
    sys.path.insert(0, "/opt/trn_rl_repo")

from contextlib import ExitStack

import numpy as np

import concourse.bacc as bacc
import concourse.bass as bass
import concourse.tile as tile
from concourse import mybir
from concourse.bass_utils import run_bass_kernel_spmd

F32 = mybir.dt.float32
EPS = 1e-8
P = 128  # steps per block == SBUF partitions

_BUILD_CACHE = {}


def build_nc(K: int, H: int, use_beta: bool, grp: int = 4) -> bass.Bass:
    """Build the single-core Bass program (SPMD across cores)."""
    NB = K // P            # number of 128-step blocks
    NG = NB // grp         # number of DMA groups (grp blocks = grp*256KiB each)
    assert NB * P == K and NG * grp == NB

    nc = bacc.Bacc("TRN2", target_bir_lowering=False, debug=False)
    x_in = nc.dram_tensor("x", [K, H], F32, kind="ExternalInput")
    gamma_in = nc.dram_tensor("gamma", [1, H], F32, kind="ExternalInput")
    beta_in = (
        nc.dram_tensor("beta", [1, H], F32, kind="ExternalInput") if use_beta else None
    )
    tri_in = nc.dram_tensor("tri", [P, P], F32, kind="ExternalInput")
    invk_in = nc.dram_tensor("invk", [P, NB], F32, kind="ExternalInput")
    out_ext = nc.dram_tensor("out", [K, H], F32, kind="ExternalOutput")

    x_ap = x_in.ap()
    out_ap = out_ext.ap()

    with tile.TileContext(nc) as tc, ExitStack() as ctx:
        singles = ctx.enter_context(tc.tile_pool(name="singles", bufs=1))
        xpool = ctx.enter_context(tc.tile_pool(name="x", bufs=NG))
        bnpool = ctx.enter_context(tc.tile_pool(name="bn", bufs=4))
        psum = ctx.enter_context(tc.tile_pool(name="psum", bufs=1, space="PSUM"))

        # --- constants ---
        gamma_b = singles.tile([P, H], F32)
        g_ap = gamma_in.ap()
        nc.sync.dma_start(
            out=gamma_b,
            in_=bass.AP(tensor=g_ap.tensor, offset=g_ap.offset, ap=[[0, P], g_ap.ap[1]]),
        )
        if use_beta:
            beta_b = singles.tile([P, H], F32)
            b_ap = beta_in.ap()
            nc.sync.dma_start(
                out=beta_b,
                in_=bass.AP(
                    tensor=b_ap.tensor, offset=b_ap.offset, ap=[[0, P], b_ap.ap[1]]
                ),
            )
        tri_sb = singles.tile([P, P], F32)
        nc.sync.dma_start(out=tri_sb, in_=tri_in.ap())
        invk_sb = singles.tile([P, NB], F32)
        nc.sync.dma_start(out=invk_sb, in_=invk_in.ap())
        ones1 = singles.tile([1, P], F32)
        nc.vector.memset(ones1, 1.0)
        ones_col = singles.tile([P, 1], F32)
        nc.vector.memset(ones_col, 1.0)
        eps_sb = singles.tile([P, 1], F32)
        nc.vector.memset(eps_sb, EPS)
        zrow = singles.tile([1, NB - 1], F32)
        nc.vector.memset(zrow, 0.0)

        # --- stats buffers ---
        mvall = singles.tile([P, NB, 2], F32)   # per-block (mean, var) per step
        est = singles.tile([P, NB], F32)        # per-step E[x^2]
        offrow0 = singles.tile([1, NB], F32)    # exclusive block offsets (m)
        offrow1 = singles.tile([1, NB], F32)    # exclusive block offsets (e)
        nc.vector.memset(offrow0, 0.0)
        nc.vector.memset(offrow1, 0.0)
        meanall = singles.tile([P, NB], F32)
        qall = singles.tile([P, NB], F32)
        msq = singles.tile([P, NB], F32)
        varall = singles.tile([P, NB], F32)
        stdall = singles.tile([P, NB], F32)
        istdall = singles.tile([P, NB], F32)

        # --- load all groups; per-block stats ---
        xg = []
        for g in range(NG):
            xt = xpool.tile([P, grp, H], F32)
            xg.append(xt)
            src = x_ap[g * grp * P : (g + 1) * grp * P, :].rearrange(
                "(t p) h -> p t h", p=P
            )
            nc.sync.dma_start(out=xt, in_=src)

        for t in range(NB):
            g, tl = divmod(t, grp)
            bnout = bnpool.tile([P, 6], F32)
            nc.vector.bn_stats(out=bnout, in_=xg[g][:, tl, :])
            nc.vector.bn_aggr(out=mvall[:, t, :], in_=bnout)
            # e = mean*mean + var
            nc.vector.tensor_scalar(
                out=est[:, t : t + 1],
                in0=mvall[:, t, 0:1],
                scalar1=mvall[:, t, 0:1],
                scalar2=mvall[:, t, 1:2],
                op0=mybir.AluOpType.mult,
                op1=mybir.AluOpType.add,
            )

        # --- cumulative sums over steps ---
        cumS = psum.tile([P, NB], F32)
        cumQ = psum.tile([P, NB], F32)
        nc.tensor.matmul(cumS, lhsT=tri_sb, rhs=mvall[:, :, 0], start=True, stop=True)
        nc.tensor.matmul(cumQ, lhsT=tri_sb, rhs=est, start=True, stop=True)
        # block totals at partition 0 (row 127 of cumS is not a legal AP base)
        tot0 = psum.tile([1, NB], F32)
        tot1 = psum.tile([1, NB], F32)
        nc.tensor.matmul(tot0, lhsT=ones_col, rhs=mvall[:, :, 0], start=True, stop=True)
        nc.tensor.matmul(tot1, lhsT=ones_col, rhs=est, start=True, stop=True)
        # exclusive scan of block totals -> offrow[:, 1:]
        nc.vector.tensor_tensor_scan(
            out=offrow0[0:1, 1:NB],
            data0=tot0[0:1, 0 : NB - 1],
            data1=zrow,
            initial=0.0,
            op0=mybir.AluOpType.add,
            op1=mybir.AluOpType.add,
        )
        nc.vector.tensor_tensor_scan(
            out=offrow1[0:1, 1:NB],
            data0=tot1[0:1, 0 : NB - 1],
            data1=zrow,
            initial=0.0,
            op0=mybir.AluOpType.add,
            op1=mybir.AluOpType.add,
        )
        # broadcast offsets across partitions, accumulate into the PSUM cumsums
        nc.tensor.matmul(cumS, lhsT=ones1, rhs=offrow0, start=False, stop=True, skip_group_check=True)
        nc.tensor.matmul(cumQ, lhsT=ones1, rhs=offrow1, start=False, stop=True, skip_group_check=True)

        # --- mean / var / istd ---
        nc.vector.tensor_mul(meanall, cumS, invk_sb)
        nc.vector.tensor_mul(qall, cumQ, invk_sb)
        nc.vector.tensor_mul(msq, meanall, meanall)
        nc.vector.tensor_sub(varall, qall, msq)
        nc.scalar.activation(
            out=stdall,
            in_=varall,
            func=mybir.ActivationFunctionType.Sqrt,
            bias=eps_sb,
            scale=1.0,
        )
        nc.vector.reciprocal(istdall, stdall)

        # --- normalize + scale, store ---
        for t in range(NB):
            g, tl = divmod(t, grp)
            nc.vector.tensor_scalar(
                out=xg[g][:, tl, :],
                in0=xg[g][:, tl, :],
                scalar1=meanall[:, t : t + 1],
                scalar2=istdall[:, t : t + 1],
                op0=mybir.AluOpType.subtract,
                op1=mybir.AluOpType.mult,
            )
            nc.gpsimd.tensor_mul(xg[g][:, tl, :], xg[g][:, tl, :], gamma_b)
            if use_beta:
                nc.vector.tensor_add(xg[g][:, tl, :], xg[g][:, tl, :], beta_b)
            if tl == grp - 1:
                dst = out_ap[g * grp * P : (g + 1) * grp * P, :].rearrange(
                    "(t p) h -> p t h", p=P
                )
                nc.sync.dma_start(out=dst, in_=xg[g])

    nc.finalize()
    return nc


def host_constants(K: int) -> dict[str, np.ndarray]:
    NB = K // P
    tri = np.triu(np.ones((P, P), dtype=np.float32))  # tri[q,p] = 1 iff q <= p
    j = np.arange(P, dtype=np.float32)[:, None]
    t = np.arange(NB, dtype=np.float32)[None, :]
    invk = (1.0 / (t * P + j + 1.0)).astype(np.float32)
    return {"tri": tri, "invk": invk}


def _get_nc(K: int, H: int, use_beta: bool) -> bass.Bass:
    key = (K, H, use_beta)
    if key not in _BUILD_CACHE:
        _BUILD_CACHE[key] = build_nc(K, H, use_beta)
    return _BUILD_CACHE[key]


def kernel(inputs: np.ndarray, gamma: np.ndarray, beta: np.ndarray) -> np.ndarray:
    B, K, H = inputs.shape
    n_cores = 8
    assert B == n_cores, f"expected B == 8, got {B}"
    use_beta = bool(np.any(beta != 0))
    nc = _get_nc(K, H, use_beta)

    consts = host_constants(K)
    gamma2 = np.ascontiguousarray(gamma, dtype=np.float32).reshape(1, H)
    beta2 = np.ascontiguousarray(beta, dtype=np.float32).reshape(1, H)
    in_maps = []
    for i in range(n_cores):
        m = {
            "x": np.ascontiguousarray(inputs[i], dtype=np.float32),
            "gamma": gamma2,
            "tri": consts["tri"],
            "invk": consts["invk"],
        }
        if use_beta:
            m["beta"] = beta2
        in_maps.append(m)

    res = run_bass_kernel_spmd(nc, in_maps, list(range(n_cores)))
    out = np.stack([res.results[i]["out"] for i in range(n_cores)], axis=0)
    return out.astype(np.float32)


# revision 12
# speedup vs baseline: 1.1060x; 1.1060x over previous
"""Cumulative LayerNorm Trainium2 kernel.

For each step k, normalize inputs[:, k] by the mean/var of the prefix
inputs[:, :k+1] over both time and feature axes, then scale/shift by
gamma/beta.

Sharding: data-parallel over batch B=8 across 8 NeuronCores (one batch
element per core); gamma/beta replicated.

Per-core algorithm (K=8192 steps, H=512 features):
  - 64 blocks of [128 steps x 512 feat].  bn_stats/bn_aggr give per-step
    mean m_k and var v_k over H; e_k = v_k + m_k^2 = E[x^2]_k.
  - cumsum over k of m and e:  within-block prefix via a triangular
    matmul (contracts the partition axis), cross-block offsets via a
    tensor_tensor_scan over the block-totals row, broadcast back across
    partitions with a rank-1 ones matmul accumulated into the same PSUM.
  - mean_k = cumsum(m)[k] / (k+1);  q_k = cumsum(e)[k] / (k+1);
    var_k = q_k - mean_k^2;  istd = 1/sqrt(var_k + eps).
  - out = gamma * (x - mean) * istd (+ beta), fused as a per-partition
    tensor_scalar plus a broadcast tensor_tensor multiply.
"""

import sys

if "/opt/trn_rl_repo" not in sys.path:
    sys.path.insert(0, "/opt/trn_rl_repo")

from contextlib import ExitStack

import numpy as np

import concourse.bacc as bacc
import concourse.bass as bass
import concourse.tile as tile
from concourse import mybir
from concourse.bass_utils import run_bass_kernel_spmd

F32 = mybir.dt.float32
EPS = 1e-8
P = 128  # steps per block == SBUF partitions

_BUILD_CACHE = {}


def build_nc(K: int, H: int, use_beta: bool, grp: int = 4) -> bass.Bass:
    """Build the single-core Bass program (SPMD across cores)."""
    NB = K // P            # number of 128-step blocks
    NG = NB // grp         # number of DMA groups (grp blocks = grp*256KiB each)
    assert NB * P == K and NG * grp == NB

    nc = bacc.Bacc("TRN2", target_bir_lowering=False, debug=False)
    x_in = nc.dram_tensor("x", [K, H], F32, kind="ExternalInput")
    gamma_in = nc.dram_tensor("gamma", [1, H], F32, kind="ExternalInput")
    beta_in = (
        nc.dram_tensor("beta", [1, H], F32, kind="ExternalInput") if use_beta else None
    )
    tri_in = nc.dram_tensor("tri", [P, P], F32, kind="ExternalInput")
    invk_in = nc.dram_tensor("invk", [P, NB], F32, kind="ExternalInput")
    out_ext = nc.dram_tensor("out", [K, H], F32, kind="ExternalOutput")

    x_ap = x_in.ap()
    out_ap = out_ext.ap()

    with tile.TileContext(nc) as tc, ExitStack() as ctx:
        singles = ctx.enter_context(tc.tile_pool(name="singles", bufs=1))
        xpool = ctx.enter_context(tc.tile_pool(name="x", bufs=NG))
        bnpool = ctx.enter_context(tc.tile_pool(name="bn", bufs=4))
        psum = ctx.enter_context(tc.tile_pool(name="psum", bufs=1, space="PSUM"))

        # --- constants ---
        gamma_b = singles.tile([P, H], F32)
        g_ap = gamma_in.ap()
        nc.sync.dma_start(
            out=gamma_b,
            in_=bass.AP(tensor=g_ap.tensor, offset=g_ap.offset, ap=[[0, P], g_ap.ap[1]]),
        )
        if use_beta:
            beta_b = singles.tile([P, H], F32)
            b_ap = beta_in.ap()
            nc.sync.dma_start(
                out=beta_b,
                in_=bass.AP(
                    tensor=b_ap.tensor, offset=b_ap.offset, ap=[[0, P], b_ap.ap[1]]
                ),
            )
        tri_sb = singles.tile([P, P], F32)
        nc.sync.dma_start(out=tri_sb, in_=tri_in.ap())
        invk_sb = singles.tile([P, NB], F32)
        nc.sync.dma_start(out=invk_sb, in_=invk_in.ap())
        ones1 = singles.tile([1, P], F32)
        nc.vector.memset(ones1, 1.0)
        ones_col = singles.tile([P, 1], F32)
        nc.vector.memset(ones_col, 1.0)
        eps_sb = singles.tile([P, 1], F32)
        nc.vector.memset(eps_sb, EPS)
        zrow = singles.tile([1, NB - 1], F32)
        nc.vector.memset(zrow, 0.0)

        # --- stats buffers ---
        mvall = singles.tile([P, NB, 2], F32)   # per-block (mean, var) per step
        est = singles.tile([P, NB], F32)        # per-step E[x^2]
        offrow0 = singles.tile([1, NB], F32)    # exclusive block offsets (m)
        offrow1 = singles.tile([1, NB], F32)    # exclusive block offsets (e)
        nc.vector.memset(offrow0, 0.0)
        nc.vector.memset(offrow1, 0.0)
        meanall = singles.tile([P, NB], F32)
        qall = singles.tile([P, NB], F32)
        msq = singles.tile([P, NB], F32)
        varall = singles.tile([P, NB], F32)
        stdall = singles.tile([P, NB], F32)
        istdall = singles.tile([P, NB], F32)
        nmi = singles.tile([P, NB], F32)        # -mean * istd (affine bias)

        # --- load all groups; per-block stats ---
        xg = []
        for g in range(NG):
            xt = xpool.tile([P, grp, H], F32)
            xg.append(xt)
            src = x_ap[g * grp * P : (g + 1) * grp * P, :].rearrange(
                "(t p) h -> p t h", p=P
            )
            nc.sync.dma_start(out=xt, in_=src)

        for t in range(NB):
            g, tl = divmod(t, grp)
            bnout = bnpool.tile([P, 6], F32)
            nc.vector.bn_stats(out=bnout, in_=xg[g][:, tl, :])
            nc.vector.bn_aggr(out=mvall[:, t, :], in_=bnout)
            # e = mean*mean + var
            nc.vector.tensor_scalar(
                out=est[:, t : t + 1],
                in0=mvall[:, t, 0:1],
                scalar1=mvall[:, t, 0:1],
                scalar2=mvall[:, t, 1:2],
                op0=mybir.AluOpType.mult,
                op1=mybir.AluOpType.add,
            )

        # --- cumulative sums over steps ---
        cumS = psum.tile([P, NB], F32)
        cumQ = psum.tile([P, NB], F32)
        nc.tensor.matmul(cumS, lhsT=tri_sb, rhs=mvall[:, :, 0], start=True, stop=True)
        nc.tensor.matmul(cumQ, lhsT=tri_sb, rhs=est, start=True, stop=True)
        # block totals at partition 0 (row 127 of cumS is not a legal AP base)
        tot0 = psum.tile([1, NB], F32)
        tot1 = psum.tile([1, NB], F32)
        nc.tensor.matmul(tot0, lhsT=ones_col, rhs=mvall[:, :, 0], start=True, stop=True)
        nc.tensor.matmul(tot1, lhsT=ones_col, rhs=est, start=True, stop=True)
        # exclusive scan of block totals -> offrow[:, 1:]
        nc.vector.tensor_tensor_scan(
            out=offrow0[0:1, 1:NB],
            data0=tot0[0:1, 0 : NB - 1],
            data1=zrow,
            initial=0.0,
            op0=mybir.AluOpType.add,
            op1=mybir.AluOpType.add,
        )
        nc.vector.tensor_tensor_scan(
            out=offrow1[0:1, 1:NB],
            data0=tot1[0:1, 0 : NB - 1],
            data1=zrow,
            initial=0.0,
            op0=mybir.AluOpType.add,
            op1=mybir.AluOpType.add,
        )
        # broadcast offsets across partitions, accumulate into the PSUM cumsums
        nc.tensor.matmul(cumS, lhsT=ones1, rhs=offrow0, start=False, stop=True, skip_group_check=True)
        nc.tensor.matmul(cumQ, lhsT=ones1, rhs=offrow1, start=False, stop=True, skip_group_check=True)

        # --- mean / var / istd ---
        nc.vector.tensor_mul(meanall, cumS, invk_sb)
        nc.vector.tensor_mul(qall, cumQ, invk_sb)
        nc.vector.tensor_mul(msq, meanall, meanall)
        nc.vector.tensor_sub(varall, qall, msq)
        nc.scalar.activation(
            out=stdall,
            in_=varall,
            func=mybir.ActivationFunctionType.Sqrt,
            bias=eps_sb,
            scale=1.0,
        )
        nc.vector.reciprocal(istdall, stdall)
        nc.vector.tensor_mul(nmi, meanall, istdall)
        nc.vector.tensor_scalar_mul(nmi, nmi, -1.0)

        # --- normalize + scale, store ---
        # affine (x*istd - mean*istd) on the otherwise-idle Scalar engine;
        # gamma multiply split DVE/GpSimd to balance engine load.
        for t in range(NB):
            g, tl = divmod(t, grp)
            nc.scalar.activation(
                out=xg[g][:, tl, :],
                in_=xg[g][:, tl, :],
                func=mybir.ActivationFunctionType.Identity,
                bias=nmi[:, t : t + 1],
                scale=istdall[:, t : t + 1],
            )
            geng = nc.vector if t % 8 == 0 else nc.gpsimd
            geng.tensor_mul(xg[g][:, tl, :], xg[g][:, tl, :], gamma_b)
            if use_beta:
                nc.vector.tensor_add(xg[g][:, tl, :], xg[g][:, tl, :], beta_b)
            if tl == grp - 1:
                dst = out_ap[g * grp * P : (g + 1) * grp * P, :].rearrange(
                    "(t p) h -> p t h", p=P
                )
                nc.sync.dma_start(out=dst, in_=xg[g])

    nc.finalize()
    return nc


def host_constants(K: int) -> dict[str, np.ndarray]:
    NB = K // P
    tri = np.triu(np.ones((P, P), dtype=np.float32))  # tri[q,p] = 1 iff q <= p
    j = np.arange(P, dtype=np.float32)[:, None]
    t = np.arange(NB, dtype=np.float32)[None, :]
    invk = (1.0 / (t * P + j + 1.0)).astype(np.float32)
    return {"tri": tri, "invk": invk}


def _get_nc(K: int, H: int, use_beta: bool) -> bass.Bass:
    key = (K, H, use_beta)
    if key not in _BUILD_CACHE:
        _BUILD_CACHE[key] = build_nc(K, H, use_beta)
    return _BUILD_CACHE[key]


def kernel(inputs: np.ndarray, gamma: np.ndarray, beta: np.ndarray) -> np.ndarray:
    B, K, H = inputs.shape
    n_cores = 8
    assert B == n_cores, f"expected B == 8, got {B}"
    use_beta = bool(np.any(beta != 0))
    nc = _get_nc(K, H, use_beta)

    consts = host_constants(K)
    gamma2 = np.ascontiguousarray(gamma, dtype=np.float32).reshape(1, H)
    beta2 = np.ascontiguousarray(beta, dtype=np.float32).reshape(1, H)
    in_maps = []
    for i in range(n_cores):
        m = {
            "x": np.ascontiguousarray(inputs[i], dtype=np.float32),
            "gamma": gamma2,
            "tri": consts["tri"],
            "invk": consts["invk"],
        }
        if use_beta:
            m["beta"] = beta2
        in_maps.append(m)

    res = run_bass_kernel_spmd(nc, in_maps, list(range(n_cores)))
    out = np.stack([res.results[i]["out"] for i in range(n_cores)], axis=0)
    return out.astype(np.float32)


# revision 14
# speedup vs baseline: 1.2037x; 1.0883x over previous
"""Cumulative LayerNorm Trainium2 kernel.

For each step k, normalize inputs[:, k] by the mean/var of the prefix
inputs[:, :k+1] over both time and feature axes, then scale/shift by
gamma/beta.

Sharding: data-parallel over batch B=8 across 8 NeuronCores (one batch
element per core); gamma/beta replicated.

Per-core algorithm (K=8192 steps, H=512 features):
  - 64 blocks of [128 steps x 512 feat].  bn_stats/bn_aggr give per-step
    mean m_k and var v_k over H; e_k = v_k + m_k^2 = E[x^2]_k.
  - cumsum over k of m and e:  within-block prefix via a triangular
    matmul (contracts the partition axis), cross-block offsets via a
    tensor_tensor_scan over the block-totals row, broadcast back across
    partitions with a rank-1 ones matmul accumulated into the same PSUM.
  - mean_k = cumsum(m)[k] / (k+1);  q_k = cumsum(e)[k] / (k+1);
    var_k = q_k - mean_k^2;  istd = 1/sqrt(var_k + eps).
  - out = gamma * (x - mean) * istd (+ beta), fused as a per-partition
    tensor_scalar plus a broadcast tensor_tensor multiply.
"""

import sys

if "/opt/trn_rl_repo" not in sys.path:
    sys.path.insert(0, "/opt/trn_rl_repo")

from contextlib import ExitStack

import numpy as np

import concourse.bacc as bacc
import concourse.bass as bass
import concourse.tile as tile
from concourse import mybir
from concourse.bass_utils import run_bass_kernel_spmd

F32 = mybir.dt.float32
EPS = 1e-8
P = 128  # steps per block == SBUF partitions

_BUILD_CACHE = {}


def build_nc(K: int, H: int, use_beta: bool, grp: int = 4) -> bass.Bass:
    """Build the single-core Bass program (SPMD across cores)."""
    NB = K // P            # number of 128-step blocks
    NG = NB // grp         # number of DMA groups (grp blocks = grp*256KiB each)
    assert NB * P == K and NG * grp == NB

    nc = bacc.Bacc("TRN2", target_bir_lowering=False, debug=False)
    x_in = nc.dram_tensor("x", [K, H], F32, kind="ExternalInput")
    gamma_in = nc.dram_tensor("gamma", [1, H], F32, kind="ExternalInput")
    beta_in = (
        nc.dram_tensor("beta", [1, H], F32, kind="ExternalInput") if use_beta else None
    )
    tri_in = nc.dram_tensor("tri", [P, P], F32, kind="ExternalInput")
    invk_in = nc.dram_tensor("invk", [P, NB], F32, kind="ExternalInput")
    out_ext = nc.dram_tensor("out", [K, H], F32, kind="ExternalOutput")

    x_ap = x_in.ap()
    out_ap = out_ext.ap()

    with tile.TileContext(nc) as tc, ExitStack() as ctx:
        singles = ctx.enter_context(tc.tile_pool(name="singles", bufs=1))
        xpool = ctx.enter_context(tc.tile_pool(name="x", bufs=NG))
        bnpool = ctx.enter_context(tc.tile_pool(name="bn", bufs=4))
        psum = ctx.enter_context(tc.tile_pool(name="psum", bufs=1, space="PSUM"))

        # --- constants ---
        gamma_b = singles.tile([P, H], F32)
        g_ap = gamma_in.ap()
        nc.sync.dma_start(
            out=gamma_b,
            in_=bass.AP(tensor=g_ap.tensor, offset=g_ap.offset, ap=[[0, P], g_ap.ap[1]]),
        )
        if use_beta:
            beta_b = singles.tile([P, H], F32)
            b_ap = beta_in.ap()
            nc.sync.dma_start(
                out=beta_b,
                in_=bass.AP(
                    tensor=b_ap.tensor, offset=b_ap.offset, ap=[[0, P], b_ap.ap[1]]
                ),
            )
        tri_sb = singles.tile([P, P], F32)
        nc.sync.dma_start(out=tri_sb, in_=tri_in.ap())
        invk_sb = singles.tile([P, NB], F32)
        nc.sync.dma_start(out=invk_sb, in_=invk_in.ap())
        ones1 = singles.tile([1, P], F32)
        nc.vector.memset(ones1, 1.0)
        ones_col = singles.tile([P, 1], F32)
        nc.vector.memset(ones_col, 1.0)
        eps_sb = singles.tile([P, 1], F32)
        nc.vector.memset(eps_sb, EPS)
        zrow = singles.tile([1, NB - 1], F32)
        nc.vector.memset(zrow, 0.0)

        # --- stats buffers ---
        mvall = singles.tile([P, NB, 2], F32)   # per-block (mean, var) per step
        est = singles.tile([P, NB], F32)        # per-step E[x^2]
        offrow0 = singles.tile([1, NB], F32)    # exclusive block offsets (m)
        offrow1 = singles.tile([1, NB], F32)    # exclusive block offsets (e)
        nc.vector.memset(offrow0, 0.0)
        nc.vector.memset(offrow1, 0.0)
        meanall = singles.tile([P, NB], F32)
        qall = singles.tile([P, NB], F32)
        msq = singles.tile([P, NB], F32)
        varall = singles.tile([P, NB], F32)
        stdall = singles.tile([P, NB], F32)
        istdall = singles.tile([P, NB], F32)
        nmi = singles.tile([P, NB], F32)        # -mean * istd (affine bias)

        # --- load all groups; per-block stats ---
        xg = []
        for g in range(NG):
            xt = xpool.tile([P, grp, H], F32)
            xg.append(xt)
            src = x_ap[g * grp * P : (g + 1) * grp * P, :].rearrange(
                "(t p) h -> p t h", p=P
            )
            nc.sync.dma_start(out=xt, in_=src)

        for t in range(NB):
            g, tl = divmod(t, grp)
            bnout = bnpool.tile([P, 6], F32)
            nc.vector.bn_stats(out=bnout, in_=xg[g][:, tl, :])
            nc.vector.bn_aggr(out=mvall[:, t, :], in_=bnout)
        # e = mean*mean + var, batched over all blocks
        nc.vector.tensor_mul(est, mvall[:, :, 0], mvall[:, :, 0])
        nc.vector.tensor_add(est, est, mvall[:, :, 1])

        # --- cumulative sums over steps ---
        cumS = psum.tile([P, NB], F32)
        cumQ = psum.tile([P, NB], F32)
        nc.tensor.matmul(cumS, lhsT=tri_sb, rhs=mvall[:, :, 0], start=True, stop=True)
        nc.tensor.matmul(cumQ, lhsT=tri_sb, rhs=est, start=True, stop=True)
        # block totals at partition 0 (row 127 of cumS is not a legal AP base)
        tot0 = psum.tile([1, NB], F32)
        tot1 = psum.tile([1, NB], F32)
        nc.tensor.matmul(tot0, lhsT=ones_col, rhs=mvall[:, :, 0], start=True, stop=True)
        nc.tensor.matmul(tot1, lhsT=ones_col, rhs=est, start=True, stop=True)
        # exclusive scan of block totals -> offrow[:, 1:]
        nc.vector.tensor_tensor_scan(
            out=offrow0[0:1, 1:NB],
            data0=tot0[0:1, 0 : NB - 1],
            data1=zrow,
            initial=0.0,
            op0=mybir.AluOpType.add,
            op1=mybir.AluOpType.add,
        )
        nc.vector.tensor_tensor_scan(
            out=offrow1[0:1, 1:NB],
            data0=tot1[0:1, 0 : NB - 1],
            data1=zrow,
            initial=0.0,
            op0=mybir.AluOpType.add,
            op1=mybir.AluOpType.add,
        )
        # broadcast offsets across partitions, accumulate into the PSUM cumsums
        nc.tensor.matmul(cumS, lhsT=ones1, rhs=offrow0, start=False, stop=True, skip_group_check=True)
        nc.tensor.matmul(cumQ, lhsT=ones1, rhs=offrow1, start=False, stop=True, skip_group_check=True)

        # --- mean / var / istd ---
        nc.vector.tensor_mul(meanall, cumS, invk_sb)
        nc.vector.tensor_mul(qall, cumQ, invk_sb)
        nc.vector.tensor_mul(msq, meanall, meanall)
        nc.vector.tensor_sub(varall, qall, msq)
        nc.scalar.activation(
            out=stdall,
            in_=varall,
            func=mybir.ActivationFunctionType.Sqrt,
            bias=eps_sb,
            scale=1.0,
        )
        nc.vector.reciprocal(istdall, stdall)
        nc.vector.tensor_mul(nmi, meanall, istdall)
        nc.vector.tensor_scalar_mul(nmi, nmi, -1.0)

        # --- normalize + scale, store ---
        # affine (x*istd - mean*istd) on the otherwise-idle Scalar engine;
        # gamma multiply split DVE/GpSimd to balance engine load.
        for t in range(NB):
            g, tl = divmod(t, grp)
            if t % 4 == 3:
                # fused (x - mean) * istd on DVE for a quarter of the tiles
                nc.vector.tensor_scalar(
                    out=xg[g][:, tl, :],
                    in0=xg[g][:, tl, :],
                    scalar1=meanall[:, t : t + 1],
                    scalar2=istdall[:, t : t + 1],
                    op0=mybir.AluOpType.subtract,
                    op1=mybir.AluOpType.mult,
                )
            else:
                nc.scalar.activation(
                    out=xg[g][:, tl, :],
                    in_=xg[g][:, tl, :],
                    func=mybir.ActivationFunctionType.Identity,
                    bias=nmi[:, t : t + 1],
                    scale=istdall[:, t : t + 1],
                )
            geng = nc.vector if t % 2 == 0 else nc.gpsimd
            geng.tensor_mul(xg[g][:, tl, :], xg[g][:, tl, :], gamma_b)
            if use_beta:
                nc.vector.tensor_add(xg[g][:, tl, :], xg[g][:, tl, :], beta_b)
            if tl == grp - 1:
                dst = out_ap[g * grp * P : (g + 1) * grp * P, :].rearrange(
                    "(t p) h -> p t h", p=P
                )
                nc.sync.dma_start(out=dst, in_=xg[g])

    nc.finalize()
    return nc


def host_constants(K: int) -> dict[str, np.ndarray]:
    NB = K // P
    tri = np.triu(np.ones((P, P), dtype=np.float32))  # tri[q,p] = 1 iff q <= p
    j = np.arange(P, dtype=np.float32)[:, None]
    t = np.arange(NB, dtype=np.float32)[None, :]
    invk = (1.0 / (t * P + j + 1.0)).astype(np.float32)
    return {"tri": tri, "invk": invk}


def _get_nc(K: int, H: int, use_beta: bool) -> bass.Bass:
    key = (K, H, use_beta)
    if key not in _BUILD_CACHE:
        _BUILD_CACHE[key] = build_nc(K, H, use_beta)
    return _BUILD_CACHE[key]


def kernel(inputs: np.ndarray, gamma: np.ndarray, beta: np.ndarray) -> np.ndarray:
    B, K, H = inputs.shape
    n_cores = 8
    assert B == n_cores, f"expected B == 8, got {B}"
    use_beta = bool(np.any(beta != 0))
    nc = _get_nc(K, H, use_beta)

    consts = host_constants(K)
    gamma2 = np.ascontiguousarray(gamma, dtype=np.float32).reshape(1, H)
    beta2 = np.ascontiguousarray(beta, dtype=np.float32).reshape(1, H)
    in_maps = []
    for i in range(n_cores):
        m = {
            "x": np.ascontiguousarray(inputs[i], dtype=np.float32),
            "gamma": gamma2,
            "tri": consts["tri"],
            "invk": consts["invk"],
        }
        if use_beta:
            m["beta"] = beta2
        in_maps.append(m)

    res = run_bass_kernel_spmd(nc, in_maps, list(range(n_cores)))
    out = np.stack([res.results[i]["out"] for i in range(n_cores)], axis=0)
    return out.astype(np.float32)
